# revision 1
# baseline (speedup 1.0000x reference)
"""Trainium2 Bass kernel for nn_DAInsHead (moe_routing).

Per-row hard-routed 3-layer MLP: rows with levels[i]==l get
    out[i] = W3[l].T @ relu(W2[l].T @ relu(W1[l].T @ x[i] + b1[l]) + b2[l]) + b3[l]

Strategy (vs the reference's dense 4x-redundant masked compute):
  * Host: stable-sort rows by level, deal each level's rows evenly to the 8
    cores, pad each (core, level) segment to a shared per-level capacity
    (multiple of 128, decomposed into row tiles of >=256), and transpose to
    feature-major xT [D, R_core] so the device needs no on-chip transposes.
    All matmul operands are cast to bf16 on the host (total rel err ~4e-3,
    well under the 2e-2 gate; bf16 streams at 1 cyc/row like f32r but gets
    fast FWL weight loads, which f32r-class 4-byte weights do not).
  * Device (identical SPMD program on 8 cores): for each level, keep that
    level's W1/W2 resident in SBUF and stream row tiles of 512: L1/L2 are
    K=8-chunk accumulated 128x128x512 bf16 matmuls (1 cycle/row) with
    relu+bias evictions split across DVE and ACT so PSUM banks recycle
    fast; L3 (matvec, M=1) is col-tiled into 32-col strips 0/32/64 with an
    independent start&stop matmul per K-chunk (chain-free — PSUM RAW
    accumulation waits the full drain), partial rows DMAed out per chunk.
  * Host: sum the per-chunk L3 partials, add b3, scatter per-core outputs
    back to original row order.

Measured on 8xTRN2 (this problem's shapes): 1038us (f32r baseline) ->
968us; PE busy ~96%, matmul issue pace ~216ns per 128x128x512 = the warm
2.4GHz roofline. Known non-wins (measured): fp8 DoubleRow needs 3-term
error compensation to pass 2e-2 which costs more matmuls than it saves;
mid-tile L3 interleave stalls the in-order PE behind fresh evictions.
"""
import os
import sys

sys.path.insert(0, "/opt/trn_rl_repo")

import ml_dtypes
import numpy as np

import concourse.bacc as bacc
import concourse.mybir as mybir
import concourse.tile as tile
from concourse.bass_utils import run_bass_kernel_spmd

F32 = mybir.dt.float32
F32R = mybir.dt.float32r
BF16 = mybir.dt.bfloat16
ADD = mybir.AluOpType.add
MAX = mybir.AluOpType.max
RELU = mybir.ActivationFunctionType.Relu

NC = 8          # cores
L = 4           # levels
D = 1024        # in features
H = 1024        # hidden
KC = D // 128   # contraction chunks

LAST_RESULTS = None       # BassKernelResults of the most recent run (for test.py)
_PROGRAM_CACHE = {}


def _row_tiles(c):
    """Split a per-level capacity (multiple of 128, >=256) into row-tile
    sizes, all >=256 (large moving dims amortize per-matmul overheads)."""
    tiles = [512] * (c // 512)
    rem = c % 512
    if rem == 128:
        # replace a 512 tile with 384 + 256 to keep every tile >= 256
        tiles[-1:] = [384, 256]
    elif rem:
        tiles.append(rem)
    return tiles


def _build_program(caps):
    """Build + compile the SPMD program for per-level capacities `caps`."""
    r_core = sum(caps)
    nc = bacc.Bacc("TRN2", target_bir_lowering=False, debug=False, num_devices=NC)
    xT = nc.dram_tensor("xT", [D, r_core], BF16, kind="ExternalInput")
    W1 = nc.dram_tensor("W1", [L, D, H], BF16, kind="ExternalInput")
    W2 = nc.dram_tensor("W2", [L, H, H], BF16, kind="ExternalInput")
    W3 = nc.dram_tensor("W3", [L, H, 1], BF16, kind="ExternalInput")
    b1 = nc.dram_tensor("b1", [L, H], F32, kind="ExternalInput")
    b2 = nc.dram_tensor("b2", [L, H], F32, kind="ExternalInput")
    out = nc.dram_tensor("out", [KC, r_core], F32, kind="ExternalOutput")

    xT_r = xT.rearrange("(kc p) r -> p kc r", p=128)  # [128, KC, r_core]

    with tile.TileContext(nc) as tc:
        with (
            tc.tile_pool(name="wpool", bufs=2) as wpool,
            tc.tile_pool(name="bpool", bufs=2) as bpool,
            tc.tile_pool(name="xpool", bufs=2) as xpool,
            tc.tile_pool(name="hpool", bufs=1) as hpool,
            tc.tile_pool(name="opool", bufs=6) as opool,
            tc.tile_pool(name="ps", bufs=8, space="PSUM") as ps,
        ):
            off = 0
            for lvl in range(L):
                cap = caps[lvl]
                if cap == 0:
                    continue
                # For level 0, issue the first row-tile's x DMA before the
                # weight DMAs so the PE can start as soon as the first weight
                # chunk lands instead of waiting behind 8.5MB of weights.
                # smallest tile first in level 0: minimizes the x bytes the
                # very first matmuls wait on
                tiles_l = _row_tiles(cap)
                if lvl == 0:
                    tiles_l = sorted(tiles_l)
                pre_x = None
                if lvl == 0:
                    rt0 = tiles_l[0]
                    pre_x = xpool.tile([128, KC, rt0], BF16, tag="x")
                    # first x chunk only — the very first matmul needs just
                    # this chunk plus w1k[0], so those two DMAs go first
                    nc.sync.dma_start(pre_x[:, 0, :], xT_r[:, 0, 0:rt0])
                w1k = []
                w2k = []
                t1 = wpool.tile([128, H], BF16, tag="w1k0")
                nc.sync.dma_start(t1[:], W1[lvl][0:128, :])
                w1k.append(t1)
                if lvl == 0:
                    for kc in range(1, KC):
                        nc.sync.dma_start(pre_x[:, kc, :], xT_r[:, kc, 0:rt0])
                # Tiny bias/W3 tiles before the bulk of W1/W2 so evictions
                # never wait behind 8MB of weight DMA.
                w3t = bpool.tile([128, KC], BF16, tag="w3")
                nc.sync.dma_start(w3t[:], W3[lvl].rearrange("(kc p) o -> p (kc o)", p=128))
                b1t = bpool.tile([128, H // 128], F32, tag="b1")
                nc.sync.dma_start(b1t[:], b1[lvl].rearrange("(mc p) -> p mc", p=128))
                b2t = bpool.tile([128, H // 128], F32, tag="b2")
                nc.sync.dma_start(b2t[:], b2[lvl].rearrange("(mc p) -> p mc", p=128))
                # Per-kc weight tiles so the first matmuls only wait on the
                # first 512KB of weight DMA, and level l+1 prefetch
                # double-buffers against level l (bufs=2 per tag).
                for kc in range(1, KC):
                    t1 = wpool.tile([128, H], BF16, tag=f"w1k{kc}")
                    nc.sync.dma_start(t1[:], W1[lvl][kc * 128:(kc + 1) * 128, :])
                    w1k.append(t1)
                for kc in range(KC):
                    t2 = wpool.tile([128, H], BF16, tag=f"w2k{kc}")
                    nc.sync.dma_start(t2[:], W2[lvl][kc * 128:(kc + 1) * 128, :])
                    w2k.append(t2)

                for ti, rt in enumerate(tiles_l):
                    if pre_x is not None and ti == 0:
                        x_t = pre_x
                    else:
                        x_t = xpool.tile([128, KC, rt], BF16, tag="x")
                        # per-kc chunk DMAs: the tile's first matmul needs
                        # only chunk 0 (128KB), and the chunks interleave
                        # with the next level's 4MB weight prefetch on the
                        # shared queue instead of queueing behind it
                        for kc in range(KC):
                            nc.sync.dma_start(x_t[:, kc, :],
                                              xT_r[:, kc, off:off + rt])

                    # L1 runs kc-outer in two 4-bank halves: the first matmul
                    # only depends on w1k[0] + x_t, so the PE ramps with the
                    # weight DMA stream instead of waiting for all of W1.
                    h1 = hpool.tile([128, H // 128, rt], BF16, tag="h1")
                    for half in range(2):
                        mcs = range(4 * half, 4 * half + 4)
                        accs = {mc: ps.tile([128, rt], F32, tag="acc", name="acc")
                                for mc in mcs}
                        for kc in range(KC):
                            for mc in mcs:
                                nc.tensor.matmul(
                                    accs[mc][:], w1k[kc][:, mc * 128:(mc + 1) * 128],
                                    x_t[:, kc, :], start=(kc == 0), stop=(kc == KC - 1))
                        # split evictions across DVE and the idle ACT engine
                        # so PSUM banks release ~2x faster (bank reuse gates
                        # the next matmul group's start); column-splitting
                        # each eviction across both engines was tried and
                        # measured +23us — the doubled op count and two
                        # writers per h chunk cost more than the latency
                        for mc in mcs:
                            if mc % 2 == 0:
                                nc.vector.tensor_scalar(
                                    h1[:, mc, :], accs[mc][:], b1t[:, mc:mc + 1], 0.0, ADD, MAX)
                            else:
                                nc.scalar.activation(
                                    h1[:, mc, :], accs[mc][:], RELU, bias=b1t[:, mc:mc + 1])

                    h2 = hpool.tile([128, H // 128, rt], BF16, tag="h2")
                    for half in range(2):
                        mcs = range(4 * half, 4 * half + 4)
                        accs = {mc: ps.tile([128, rt], F32, tag="acc", name="acc")
                                for mc in mcs}
                        for kc in range(H // 128):
                            for mc in mcs:
                                nc.tensor.matmul(
                                    accs[mc][:], w2k[kc][:, mc * 128:(mc + 1) * 128],
                                    h1[:, kc, :], start=(kc == 0), stop=(kc == H // 128 - 1))
                        for mc in mcs:
                            if mc % 2 == 0:
                                nc.vector.tensor_scalar(
                                    h2[:, mc, :], accs[mc][:], b2t[:, mc:mc + 1], 0.0, ADD, MAX)
                            else:
                                nc.scalar.activation(
                                    h2[:, mc, :], accs[mc][:], RELU, bias=b2t[:, mc:mc + 1])
                    # L3: col-tiled M=1 matmuls in 32-col strips 0/32/64
                    # (quadrant 3 at partition 96 is HW-buggy), at tile end
                    # (inserting it mid-tile stalls the in-order PE behind
                    # just-queued h2 evictions — measured +40us twice). Every
                    # kc chunk gets its own (bank, strip) slot as an
                    # independent start&stop matmul — chain-free, since
                    # accumulation chains across strips stall ~510ns per
                    # matmul (RAW on PSUM waits the full drain). Host sums
                    # the 8 partial rows + b3.
                    banks = [ps.tile([128, rt], F32, tag="acc", name="acc")
                             for _ in range(3)]
                    for kc in range(KC):
                        b, c = divmod(kc, 3)
                        nc.tensor.matmul(
                            banks[b][c * 32:c * 32 + 1, :], w3t[:, kc:kc + 1],
                            h2[:, kc, :], start=True, stop=True)
                    # engines are partition-locked (no cross-partition moves)
                    # and strided-partition APs are rejected, so evict each
                    # partial row in place (alternating DVE/ACT so banks
                    # release fast) and DMA the rows out on two queues
                    for b in range(3):
                        o_t = opool.tile([65, rt], F32, tag="o")
                        for c in range(3 if b < 2 else 2):
                            kc = b * 3 + c
                            if kc % 2 == 0:
                                nc.vector.tensor_scalar(
                                    o_t[c * 32:c * 32 + 1, :],
                                    banks[b][c * 32:c * 32 + 1, :], 0.0, None, ADD)
                            else:
                                nc.scalar.copy(o_t[c * 32:c * 32 + 1, :],
                                               banks[b][c * 32:c * 32 + 1, :])
                            q = nc.gpsimd if kc % 2 == 0 else nc.sync
                            q.dma_start(out[kc:kc + 1, off:off + rt],
                                        o_t[c * 32:c * 32 + 1, :])
                    off += rt
    nc.compile()
    return nc


def kernel(x, levels, W1, b1, W2, b2, W3, b3):
    global LAST_RESULTS
    x = np.ascontiguousarray(np.asarray(x, dtype=np.float32))
    levels = np.asarray(levels)
    n = x.shape[0]

    # --- host-side routing: sort rows by level, deal evenly to cores ---
    order = np.argsort(levels, kind="stable")
    counts = np.bincount(np.asarray(levels, dtype=np.int64), minlength=L)[:L]

    # per-level capacity shared by all cores: ceil(max per-core count / 128)*128,
    # min 256 (small moving dims waste per-matmul overhead)
    caps = []
    for lvl in range(L):
        per_core_max = -(-int(counts[lvl]) // NC)
        caps.append(max(-(-per_core_max // 128) * 128, 256) if per_core_max else 0)
    r_core = sum(caps)

    # per-core padded index lists + validity masks
    idx = np.zeros((NC, r_core), dtype=np.int64)
    valid = np.zeros((NC, r_core), dtype=bool)
    lvl_start = np.concatenate([[0], np.cumsum(counts)])
    seg_off = 0
    for lvl in range(L):
        rows = order[lvl_start[lvl]:lvl_start[lvl + 1]]
        nl = len(rows)
        q, rem = divmod(nl, NC)
        start = 0
        for c in range(NC):
            cnt = q + (1 if c < rem else 0)
            idx[c, seg_off:seg_off + cnt] = rows[start:start + cnt]
            valid[c, seg_off:seg_off + cnt] = True
            start += cnt
        seg_off += caps[lvl]

    key = tuple(caps)
    nc = _PROGRAM_CACHE.get(key)
    if nc is None:
        nc = _build_program(caps)
        _PROGRAM_CACHE[key] = nc

    in_maps = []
    for c in range(NC):
        xTc = np.ascontiguousarray(x[idx[c]].T).astype(ml_dtypes.bfloat16)  # [D, r_core]
        in_maps.append({
            "xT": xTc,
            "W1": np.asarray(W1, dtype=np.float32).astype(ml_dtypes.bfloat16),
            "W2": np.asarray(W2, dtype=np.float32).astype(ml_dtypes.bfloat16),
            "W3": np.asarray(W3, dtype=np.float32).astype(ml_dtypes.bfloat16),
            "b1": np.asarray(b1, dtype=np.float32),
            "b2": np.asarray(b2, dtype=np.float32),
        })

    trace = bool(os.environ.get("BASS_KERNEL_TRACE"))
    try:
        res = run_bass_kernel_spmd(nc, in_maps, core_ids=list(range(NC)), trace=trace)
    except Exception:
        # transient NRT_EXEC_UNIT_UNRECOVERABLE wedges have been observed to
        # clear on the next attempt
        import time
        time.sleep(5)
        res = run_bass_kernel_spmd(nc, in_maps, core_ids=list(range(NC)), trace=trace)
    LAST_RESULTS = res

    result = np.zeros((n, 1), dtype=np.float32)
    for c in range(NC):
        o = np.asarray(res.results[c]["out"], dtype=np.float32).sum(axis=0)
        result[idx[c][valid[c]], 0] = o[valid[c]]
    result += np.asarray(b3, dtype=np.float32)[np.asarray(levels, dtype=np.int64), :]
    return result

